# revision 9
# baseline (speedup 1.0000x reference)
"""GCN layer (gather -> segment-sum -> linear) on 8 TRN2 NeuronCores.

out = segment_sum(x[src], dst, N) @ W.T + b

Strategy (dst-sharded, SPMD single program):
- 10240 destination slots split 1280/core (last 240 are dummy).
- Per core, edges whose dst lands in its shard, sorted by dst, are packed
  into 128-edge blocks against a STATIC "conveyor" schedule of PSUM column
  windows (shared across cores; per-core feasibility guaranteed by the host
  packer, which pads with sentinel edges).
- On device, per block: hardware dma_gather pulls the 128 source rows
  (bf16, 256B each) from HBM into SBUF partitions; DVE builds a one-hot
  [128 edges x w] matrix by comparing the block's dst offsets against an
  iota; TensorE matmul accumulates h^T[fi, dst] into a PSUM bank.
- Per 512-dst window: copy h^T to SBUF, apply W via a fp32 matmul
  (out^T = W @ h^T), add bias, DMA out. Host concatenates + transposes.
"""

import numpy as np
import ml_dtypes

N_NODES = 10000
N_EDGES = 640000
D = 128
NCORES = 8
NPC = 1280  # dst slots per core (8 * 1280 = 10240 >= 10000)
WINDOWS = [(0, 512), (512, 512), (1024, 256)]  # (start, width) within a core
W_OH = 16  # one-hot width (columns per block's PSUM slice)
RHO = 0.88  # target block fill ratio
CHUNK = 64  # blocks per dma_gather call
SINGLE_PACKET = False


# ---------------------------------------------------------------- host packing

def _pack_core_window(dloc, s_arr, wS_arr):
    """Greedy earliest-fit of sorted local-dst values into blocks with static
    column slices [s_i, s_i+wS_i). Returns per-edge block id or None."""
    nbw = len(s_arr)
    blk = np.empty(len(dloc), np.int32)
    i = 0
    cap = 128
    for e in range(len(dloc)):
        d = dloc[e]
        while i < nbw and (cap == 0 or d >= s_arr[i] + wS_arr[i]):
            i += 1
            cap = 128
        if i >= nbw or s_arr[i] > d:
            return None
        blk[e] = i
        cap -= 1
    return blk


def _mk_sched(ww, nbw, w_oh, cums):
    """Quantile-frontier schedule: block i's slice starts no later than the
    dst where every core has at least 128*i edges below it (those edges fill
    blocks 0..i-1), clamped nondecreasing."""
    s = np.zeros(nbw, np.int32)
    for i in range(1, nbw):
        cands = [np.searchsorted(cu, 128 * i, side="right") - 1 for cu in cums]
        # coverage cap: consecutive slices must overlap or abut
        s[i] = max(s[i - 1], min(min(cands), s[i - 1] + w_oh, ww - 1))
    # tail must reach the end of the window
    if s[-1] + w_oh < ww:
        return None, None
    wS = np.minimum(w_oh, ww - s).astype(np.int32)
    return s, wS


def build_plan(src, dst):
    """Compute the shared block schedule + per-core gather/dst tables."""
    src = np.asarray(src).astype(np.int64)
    dst = np.asarray(dst).astype(np.int64)
    order = np.argsort(dst, kind="stable")
    dst_s = dst[order]
    src_s = src[order]

    core_lo = np.searchsorted(dst_s, np.arange(NCORES) * NPC, side="left")
    core_hi = np.searchsorted(dst_s, (np.arange(NCORES) + 1) * NPC, side="left")

    # per (core, window): local dst arrays and srcs
    per_cw = []  # [core][win] -> (dloc, srcs)
    for c in range(NCORES):
        lo, hi = core_lo[c], core_hi[c]
        dl = (dst_s[lo:hi] - c * NPC).astype(np.int32)
        sl = src_s[lo:hi]
        row = []
        for ws, ww in WINDOWS:
            a = np.searchsorted(dl, ws, side="left")
            b = np.searchsorted(dl, ws + ww, side="left")
            row.append(((dl[a:b] - ws).astype(np.int32), sl[a:b].astype(np.int32)))
        per_cw.append(row)

    # choose per-window NBW (shared) and verify feasibility on every core
    w_oh = W_OH
    scheds = []  # per window: (s_arr, wS_arr)
    assigns = [[None] * len(WINDOWS) for _ in range(NCORES)]
    for wi, (ws, ww) in enumerate(WINDOWS):
        maxcnt = max(len(per_cw[c][wi][0]) for c in range(NCORES))
        cums = []
        for c in range(NCORES):
            cnt = np.bincount(per_cw[c][wi][0], minlength=ww)
            cums.append(np.concatenate([[0], np.cumsum(cnt)]))
        nbw = max(int(np.ceil(maxcnt / 128.0)), int(np.ceil(ww / w_oh)))
        for _attempt in range(200):
            s_arr, wS_arr = _mk_sched(ww, nbw, w_oh, cums)
            ok = s_arr is not None
            if ok:
                for c in range(NCORES):
                    blk = _pack_core_window(per_cw[c][wi][0], s_arr, wS_arr)
                    if blk is None:
                        ok = False
                        break
                    assigns[c][wi] = blk
            if ok:
                break
            nbw = nbw + max(1, nbw // 100)
        else:
            raise RuntimeError("packing failed")
        scheds.append((s_arr, wS_arr))

    # flat block descriptor list (shared): (win_id, s, wS, last_in_window)
    blocks = []
    for wi in range(len(WINDOWS)):
        s_arr, wS_arr = scheds[wi]
        for i in range(len(s_arr)):
            blocks.append([wi, int(s_arr[i]), int(wS_arr[i]), False])
    # pad to multiple of CHUNK with sentinel blocks in the last window
    while len(blocks) % CHUNK != 0:
        blocks.append([len(WINDOWS) - 1, 0, 1, False])
    # mark last block of each window
    last_seen = {}
    for bi, bdesc in enumerate(blocks):
        last_seen[bdesc[0]] = bi
    for wi, bi in last_seen.items():
        blocks[bi][3] = True
    nblk = len(blocks)

    # per-core tables
    gidx_all = []
    drel_all = []
    for c in range(NCORES):
        gidx = np.zeros((nblk, 128), np.int16)
        drel = np.full((nblk, 128), -1.0, np.float32)
        base = 0
        for wi in range(len(WINDOWS)):
            s_arr, _ = scheds[wi]
            dloc, srcs = per_cw[c][wi]
            blk = assigns[c][wi]
            if len(dloc):
                # fill slot within block by running count
                ord2 = np.argsort(blk, kind="stable")
                bb = blk[ord2]
                slot = np.arange(len(bb)) - np.searchsorted(bb, bb, side="left")
                rows = base + bb
                gidx[rows, slot] = srcs[ord2].astype(np.int16)
                drel[rows, slot] = (dloc[ord2] - s_arr[bb]).astype(np.float32)
            base += len(s_arr)
        gidx_all.append(gidx)
        drel_all.append(drel)

    # device layouts
    in_tables = []
    for c in range(NCORES):
        flat = gidx_all[c].reshape(-1)  # block-major edge order
        nch = nblk // CHUNK
        idxt = np.zeros((128, nblk * 8), np.int16)
        for k in range(nch):
            seg = flat[k * CHUNK * 128 : (k + 1) * CHUNK * 128]
            wrapped = np.tile(seg.reshape(CHUNK * 8, 16).T, (8, 1))  # [128, C*8]
            idxt[:, k * CHUNK * 8 : (k + 1) * CHUNK * 8] = wrapped
        drt = drel_all[c].T.copy()  # [128, nblk]
        in_tables.append((idxt, drt))

    return blocks, in_tables


# ---------------------------------------------------------------- bass program

def build_program(blocks):
    import concourse.bass as bass  # noqa: F401
    import concourse.mybir as mybir
    import concourse.tile as tile
    from concourse import bacc

    nblk = len(blocks)
    nc = bacc.Bacc("TRN2", target_bir_lowering=False, num_devices=NCORES)
    bf16 = mybir.dt.bfloat16
    f32 = mybir.dt.float32

    xb = nc.declare_dram_parameter("xb", [N_NODES, D], bf16, isOutput=False)
    idxt = nc.declare_dram_parameter("idxt", [128, nblk * 8], mybir.dt.int16, isOutput=False)
    drt = nc.declare_dram_parameter("drt", [128, nblk], f32, isOutput=False)
    wt = nc.declare_dram_parameter("wt", [128, 128], f32, isOutput=False)
    bias = nc.declare_dram_parameter("bias", [128, 1], f32, isOutput=False)
    outp = nc.declare_dram_parameter("out", [128, NPC], f32, isOutput=True)

    with tile.TileContext(nc) as tc:
        with (
            tc.tile_pool(name="const", bufs=1) as const_pool,
            tc.tile_pool(name="msg", bufs=3) as msg_pool,
            tc.tile_pool(name="oh", bufs=8) as oh_pool,
            tc.tile_pool(name="hbuf", bufs=2) as hbuf_pool,
            tc.tile_pool(name="obuf", bufs=2) as obuf_pool,
            tc.tile_pool(name="ph", bufs=2, space="PSUM") as ph_pool,
            tc.tile_pool(name="po", bufs=2, space="PSUM") as po_pool,
        ):
            idx_sb = const_pool.tile([128, nblk * 8], mybir.dt.int16)
            nc.sync.dma_start(idx_sb[:], idxt[:])
            dr_sb = const_pool.tile([128, nblk], f32)
            nc.sync.dma_start(dr_sb[:], drt[:])
            wt_sb = const_pool.tile([128, 128], f32)
            nc.sync.dma_start(wt_sb[:], wt[:])
            b_sb = const_pool.tile([128, 1], f32)
            nc.sync.dma_start(b_sb[:], bias[:])
            iota_sb = const_pool.tile([128, W_OH], f32)
            nc.gpsimd.iota(
                iota_sb[:], [[1, W_OH]], channel_multiplier=0,
                allow_small_or_imprecise_dtypes=True,
            )
            zero_sb = const_pool.tile([128, 512], bf16)
            nc.vector.memset(zero_sb[:], 0)

            ph = None
            mt = None
            for c, (wi, s, wS, last) in enumerate(blocks):
                if ph is None:
                    ww = WINDOWS[wi][1]
                    ph = ph_pool.tile([128, ww], f32)
                    nc.tensor.matmul(
                        ph[:, :], lhsT=zero_sb[:, 0:128], rhs=zero_sb[:, 0:ww],
                        start=True, stop=False, skip_group_check=True,
                    )
                if c % CHUNK == 0:
                    mt = msg_pool.tile([128, CHUNK, D], bf16)
                    nc.gpsimd.dma_gather(
                        mt[:, :, :], xb[:, :],
                        idx_sb[:, c * 8 : (c + CHUNK) * 8],
                        num_idxs=CHUNK * 128, num_idxs_reg=CHUNK * 128,
                        elem_size=D, single_packet=SINGLE_PACKET,
                    )
                st = oh_pool.tile([128, W_OH], bf16)
                nc.vector.tensor_scalar(
                    st[:], iota_sb[:], dr_sb[:, c : c + 1], None,
                    op0=mybir.AluOpType.is_equal,
                )
                nc.tensor.matmul(
                    ph[:, s : s + wS], lhsT=mt[:, c % CHUNK, :], rhs=st[:, 0:wS],
                    start=False, stop=last, skip_group_check=True,
                )
                if last:
                    ws, ww = WINDOWS[wi]
                    hb = hbuf_pool.tile([128, ww], f32)
                    nc.vector.tensor_copy(hb[:], ph[:, :])
                    po = po_pool.tile([128, ww], f32)
                    nc.tensor.matmul(
                        po[:, :], lhsT=wt_sb[:], rhs=hb[:], start=True, stop=True,
                    )
                    ob = obuf_pool.tile([128, ww], f32)
                    nc.vector.tensor_scalar_add(ob[:], po[:, :], b_sb[:, 0:1])
                    nc.sync.dma_start(outp[:, ws : ws + ww], ob[:])
                    ph = None

    nc.compile()
    return nc


# ---------------------------------------------------------------- entry point

def kernel(x, src, dst, W, b):
    from concourse.bass_utils import run_bass_kernel_spmd

    blocks, in_tables = build_plan(src, dst)
    nc = build_program(blocks)

    xbf = np.asarray(x, np.float32).astype(ml_dtypes.bfloat16)
    wt = np.asarray(W, np.float32).T.copy()  # [fi, fo]
    bb = np.asarray(b, np.float32).reshape(128, 1)

    in_maps = []
    for c in range(NCORES):
        idxt, drt = in_tables[c]
        in_maps.append({"xb": xbf, "idxt": idxt, "drt": drt, "wt": wt, "bias": bb})

    res = run_bass_kernel_spmd(nc, in_maps, list(range(NCORES)))
    out_t = np.concatenate([res.results[c]["out"] for c in range(NCORES)], axis=1)
    return np.ascontiguousarray(out_t.T[:N_NODES]).astype(np.float32)


# revision 10
# speedup vs baseline: 6.8511x; 6.8511x over previous
"""GCN layer (gather -> segment-sum -> linear) on 8 TRN2 NeuronCores.

out = segment_sum(x[src], dst, N) @ W.T + b

Strategy (dst-sharded SPMD, dense-adjacency matmul):
- The message-passing step is h = A @ x with A[d, s] = #edges s->d.
  Equivalently h^T = x^T @ A^T: contract over source nodes on TensorE.
- 10240 destination slots split 1280 per core (last 240 dummy). Per core the
  host builds the dense bf16 adjacency block A_c [10112 src, 1280 dst]
  (counts are small ints, exact in bf16) and packs it in the exact tile
  order the device streams: 3 PSUM windows (512/512/256 dst) x 79 source
  slabs of 128, so every DMA is a contiguous line-rate read.
- Device: x lives in SBUF (pre-slabbed [128, 79*128] bf16, one DMA); for
  each window, 79 matmuls accumulate h^T[fi, dst] into a PSUM bank
  (lhsT = x slab [128 src, 128 fi] stationary, rhs = A tile [128 src, ww]);
  then h^T -> SBUF, out^T = W @ h^T (fp32 matmul) + bias, DMA out.
- Host concatenates the per-core [128, 1280] outputs and transposes.
"""

import numpy as np
import ml_dtypes

N_NODES = 10000
D = 128
NCORES = 8
NPC = 1280  # dst slots per core (8 * 1280 = 10240 >= 10000)
WINDOWS = [(0, 512), (512, 512), (1024, 256)]  # (start, width) within a core
NSLAB = (N_NODES + 127) // 128  # 79 source slabs
NPAD = NSLAB * 128  # 10112
AGRP = 4  # A tiles (slabs) per DMA


def build_tables(src, dst, x, W, b):
    """Per-core packed adjacency + pre-slabbed x and other device inputs."""
    src = np.asarray(src).astype(np.int64)
    dst = np.asarray(dst).astype(np.int64)

    cnt = np.zeros((NPAD, NCORES * NPC), np.uint8)
    np.add.at(cnt, (src, dst), 1)

    xs_pad = np.zeros((NPAD, D), np.float32)
    xs_pad[:N_NODES] = np.asarray(x, np.float32)
    # xs[p, s*128 + f] = x[s*128 + p, f]
    xs = np.ascontiguousarray(
        xs_pad.reshape(NSLAB, 128, D).transpose(1, 0, 2).reshape(128, NSLAB * D)
    ).astype(ml_dtypes.bfloat16)

    adjs = []
    for c in range(NCORES):
        ac = cnt[:, c * NPC : (c + 1) * NPC].astype(ml_dtypes.bfloat16)
        ac = ac.reshape(NSLAB, 128, NPC)
        parts = []
        for ws, ww in WINDOWS:
            # [NSLAB, 128, ww] in slab order, flattened
            parts.append(np.ascontiguousarray(ac[:, :, ws : ws + ww]).reshape(-1))
        adjs.append(np.concatenate(parts))

    wt = np.asarray(W, np.float32).T.copy()  # [fi, fo]
    bb = np.asarray(b, np.float32).reshape(128, 1)
    return xs, adjs, wt, bb


def build_program():
    import concourse.mybir as mybir
    import concourse.tile as tile
    from concourse import bacc

    nc = bacc.Bacc("TRN2", target_bir_lowering=False, num_devices=NCORES)
    bf16 = mybir.dt.bfloat16
    f32 = mybir.dt.float32

    adj_len = sum(128 * ww for _, ww in WINDOWS) * NSLAB
    xs_d = nc.declare_dram_parameter("xs", [128, NSLAB * D], bf16, isOutput=False)
    adj_d = nc.declare_dram_parameter("adj", [adj_len], bf16, isOutput=False)
    wt_d = nc.declare_dram_parameter("wt", [128, 128], f32, isOutput=False)
    b_d = nc.declare_dram_parameter("bias", [128, 1], f32, isOutput=False)
    out_d = nc.declare_dram_parameter("out", [128, NPC], f32, isOutput=True)

    with tile.TileContext(nc) as tc:
        with (
            tc.tile_pool(name="const", bufs=1) as const_pool,
            tc.tile_pool(name="adj", bufs=3) as adj_pool,
            tc.tile_pool(name="hbuf", bufs=2) as hbuf_pool,
            tc.tile_pool(name="obuf", bufs=2) as obuf_pool,
            tc.tile_pool(name="ph", bufs=2, space="PSUM") as ph_pool,
            tc.tile_pool(name="po", bufs=2, space="PSUM") as po_pool,
        ):
            xs = const_pool.tile([128, NSLAB * D], bf16)
            nc.sync.dma_start(xs[:], xs_d[:])
            wt_sb = const_pool.tile([128, 128], f32)
            nc.sync.dma_start(wt_sb[:], wt_d[:])
            b_sb = const_pool.tile([128, 1], f32)
            nc.sync.dma_start(b_sb[:], b_d[:])

            off = 0
            for ws, ww in WINDOWS:
                ph = ph_pool.tile([128, ww], f32)
                at = None
                for s in range(NSLAB):
                    g = s % AGRP
                    if g == 0:
                        ng = min(AGRP, NSLAB - s)
                        at = adj_pool.tile([128, AGRP, ww], bf16)
                        nc.sync.dma_start(
                            at[:, 0:ng, :],
                            adj_d[off : off + ng * 128 * ww].rearrange(
                                "(g p w) -> p g w", p=128, w=ww
                            ),
                        )
                        off += ng * 128 * ww
                    nc.tensor.matmul(
                        ph[:, :],
                        lhsT=xs[:, s * D : (s + 1) * D],
                        rhs=at[:, g, :],
                        start=(s == 0),
                        stop=(s == NSLAB - 1),
                    )
                hb = hbuf_pool.tile([128, ww], f32)
                nc.vector.tensor_copy(hb[:], ph[:, :])
                po = po_pool.tile([128, ww], f32)
                nc.tensor.matmul(po[:, :], lhsT=wt_sb[:], rhs=hb[:], start=True, stop=True)
                ob = obuf_pool.tile([128, ww], f32)
                nc.vector.tensor_scalar_add(ob[:], po[:, :], b_sb[:, 0:1])
                nc.sync.dma_start(out_d[:, ws : ws + ww], ob[:])

    nc.compile()
    return nc


_CACHED_NC = None


def kernel(x, src, dst, W, b):
    global _CACHED_NC
    from concourse.bass_utils import run_bass_kernel_spmd

    xs, adjs, wt, bb = build_tables(src, dst, x, W, b)
    if _CACHED_NC is None:
        _CACHED_NC = build_program()
    nc = _CACHED_NC

    in_maps = [
        {"xs": xs, "adj": adjs[c], "wt": wt, "bias": bb} for c in range(NCORES)
    ]
    res = run_bass_kernel_spmd(nc, in_maps, list(range(NCORES)))
    out_t = np.concatenate([res.results[c]["out"] for c in range(NCORES)], axis=1)
    return np.ascontiguousarray(out_t.T[:N_NODES]).astype(np.float32)


# revision 11
# speedup vs baseline: 9.7063x; 1.4167x over previous
"""GCN layer (gather -> segment-sum -> linear) on 8 TRN2 NeuronCores.

out = segment_sum(x[src], dst, N) @ W.T + b

Strategy (dst-sharded SPMD, dense-adjacency matmul):
- The message-passing step is h = A @ x with A[d, s] = #edges s->d.
  Equivalently h^T = x^T @ A^T: contract over source nodes on TensorE.
- 10240 destination slots split 1280 per core (last 240 dummy). Per core the
  host builds the dense bf16 adjacency block A_c [10112 src, 1280 dst]
  (counts are small ints, exact in bf16) and packs it in the exact tile
  order the device streams: 3 PSUM windows (512/512/256 dst) x 79 source
  slabs of 128, so every DMA is a contiguous line-rate read.
- Device: x lives in SBUF (pre-slabbed [128, 79*128] bf16, one DMA); for
  each window, 79 matmuls accumulate h^T[fi, dst] into a PSUM bank
  (lhsT = x slab [128 src, 128 fi] stationary, rhs = A tile [128 src, ww]);
  then h^T -> SBUF, out^T = W @ h^T (fp32 matmul) + bias, DMA out.
- Host concatenates the per-core [128, 1280] outputs and transposes.
"""

import numpy as np
import ml_dtypes

N_NODES = 10000
D = 128
NCORES = 8
NPC = 1280  # dst slots per core (8 * 1280 = 10240 >= 10000)
WINDOWS = [(0, 512), (512, 512), (1024, 256)]  # (start, width) within a core
NSLAB = (N_NODES + 127) // 128  # 79 source slabs
NPAD = NSLAB * 128  # 10112
AGRP = 8  # A tiles (slabs) per DMA


def build_tables(src, dst, x, W, b):
    """Per-core packed adjacency + pre-slabbed x and other device inputs."""
    src = np.asarray(src).astype(np.int64)
    dst = np.asarray(dst).astype(np.int64)

    cnt = np.zeros((NPAD, NCORES * NPC), np.uint8)
    np.add.at(cnt, (src, dst), 1)

    xs_pad = np.zeros((NPAD, D), np.float32)
    xs_pad[:N_NODES] = np.asarray(x, np.float32)
    # xs[p, s*128 + f] = x[s*128 + p, f]
    xs = np.ascontiguousarray(
        xs_pad.reshape(NSLAB, 128, D).transpose(1, 0, 2).reshape(128, NSLAB * D)
    ).astype(ml_dtypes.bfloat16)

    assert cnt.max() <= 15, "edge multiplicity too large for fp8e4"
    adjs = []
    for c in range(NCORES):
        ac = cnt[:, c * NPC : (c + 1) * NPC].astype(ml_dtypes.float8_e4m3)
        ac = ac.reshape(NSLAB, 128, NPC)
        parts = []
        for ws, ww in WINDOWS:
            for s0 in range(0, NSLAB, AGRP):
                ng = min(AGRP, NSLAB - s0)
                # per-partition contiguous: [p][g][w]
                blk = ac[s0 : s0 + ng, :, ws : ws + ww].transpose(1, 0, 2)
                parts.append(np.ascontiguousarray(blk).reshape(-1))
        adjs.append(np.concatenate(parts))

    wt = np.asarray(W, np.float32).T.copy()  # [fi, fo]
    bb = np.asarray(b, np.float32).reshape(128, 1)
    return xs, adjs, wt, bb


def build_program():
    import concourse.mybir as mybir
    import concourse.tile as tile
    from concourse import bacc

    nc = bacc.Bacc("TRN2", target_bir_lowering=False, num_devices=NCORES)
    bf16 = mybir.dt.bfloat16
    f32 = mybir.dt.float32

    adj_len = sum(128 * ww for _, ww in WINDOWS) * NSLAB
    xs_d = nc.declare_dram_parameter("xs", [128, NSLAB * D], bf16, isOutput=False)
    adj_d = nc.declare_dram_parameter("adj", [adj_len], mybir.dt.float8e4, isOutput=False)
    wt_d = nc.declare_dram_parameter("wt", [128, 128], f32, isOutput=False)
    b_d = nc.declare_dram_parameter("bias", [128, 1], f32, isOutput=False)
    out_d = nc.declare_dram_parameter("out", [128, NPC], f32, isOutput=True)

    with tile.TileContext(nc) as tc:
        with (
            tc.tile_pool(name="const", bufs=1) as const_pool,
            tc.tile_pool(name="adj", bufs=3) as adj_pool,
            tc.tile_pool(name="hbuf", bufs=2) as hbuf_pool,
            tc.tile_pool(name="obuf", bufs=2) as obuf_pool,
            tc.tile_pool(name="ph", bufs=2, space="PSUM") as ph_pool,
            tc.tile_pool(name="po", bufs=2, space="PSUM") as po_pool,
        ):
            xs = const_pool.tile([128, NSLAB * D], bf16)
            nc.sync.dma_start(xs[:], xs_d[:])
            wt_sb = const_pool.tile([128, 128], f32)
            nc.sync.dma_start(wt_sb[:], wt_d[:])
            b_sb = const_pool.tile([128, 1], f32)
            nc.sync.dma_start(b_sb[:], b_d[:])

            off = 0
            for ws, ww in WINDOWS:
                ph = ph_pool.tile([128, ww], f32)
                at = None
                for s in range(NSLAB):
                    g = s % AGRP
                    if g == 0:
                        ng = min(AGRP, NSLAB - s)
                        at = adj_pool.tile([128, AGRP, ww], mybir.dt.float8e4)
                        nc.sync.dma_start(
                            at[:, 0:ng, :],
                            adj_d[off : off + ng * 128 * ww].rearrange(
                                "(p g w) -> p g w", g=ng, w=ww
                            ),
                        )
                        off += ng * 128 * ww
                    nc.tensor.matmul(
                        ph[:, :],
                        lhsT=xs[:, s * D : (s + 1) * D],
                        rhs=at[:, g, :],
                        start=(s == 0),
                        stop=(s == NSLAB - 1),
                    )
                hb = hbuf_pool.tile([128, ww], f32)
                nc.vector.tensor_copy(hb[:], ph[:, :])
                po = po_pool.tile([128, ww], f32)
                nc.tensor.matmul(po[:, :], lhsT=wt_sb[:], rhs=hb[:], start=True, stop=True)
                ob = obuf_pool.tile([128, ww], f32)
                nc.vector.tensor_scalar_add(ob[:], po[:, :], b_sb[:, 0:1])
                nc.sync.dma_start(out_d[:, ws : ws + ww], ob[:])

    nc.compile()
    return nc


_CACHED_NC = None


def kernel(x, src, dst, W, b):
    global _CACHED_NC
    from concourse.bass_utils import run_bass_kernel_spmd

    xs, adjs, wt, bb = build_tables(src, dst, x, W, b)
    if _CACHED_NC is None:
        _CACHED_NC = build_program()
    nc = _CACHED_NC

    in_maps = [
        {"xs": xs, "adj": adjs[c], "wt": wt, "bias": bb} for c in range(NCORES)
    ]
    res = run_bass_kernel_spmd(nc, in_maps, list(range(NCORES)))
    out_t = np.concatenate([res.results[c]["out"] for c in range(NCORES)], axis=1)
    return np.ascontiguousarray(out_t.T[:N_NODES]).astype(np.float32)


# revision 12
# speedup vs baseline: 13.9865x; 1.4410x over previous
"""GCN layer (gather -> segment-sum -> linear) on 8 TRN2 NeuronCores.

out = segment_sum(x[src], dst, N) @ W.T + b

Strategy (dst-sharded SPMD, dense-adjacency matmul):
- The message-passing step is h = A @ x with A[d, s] = #edges s->d.
  Equivalently h^T = x^T @ A^T: contract over source nodes on TensorE.
- 10240 destination slots split 1280 per core (last 240 dummy). Per core the
  host builds the dense bf16 adjacency block A_c [10112 src, 1280 dst]
  (counts are small ints, exact in bf16) and packs it in the exact tile
  order the device streams: 3 PSUM windows (512/512/256 dst) x 79 source
  slabs of 128, so every DMA is a contiguous line-rate read.
- Device: x lives in SBUF (pre-slabbed [128, 79*128] bf16, one DMA); for
  each window, 79 matmuls accumulate h^T[fi, dst] into a PSUM bank
  (lhsT = x slab [128 src, 128 fi] stationary, rhs = A tile [128 src, ww]);
  then h^T -> SBUF, out^T = W @ h^T (fp32 matmul) + bias, DMA out.
- Host concatenates the per-core [128, 1280] outputs and transposes.
"""

import numpy as np
import ml_dtypes

N_NODES = 10000
D = 128
NCORES = 8
NPC = 1280  # dst slots per core (8 * 1280 = 10240 >= 10000)
WINDOWS = [(0, 512), (512, 512), (1024, 256)]  # (start, width) within a core
NSLAB = (N_NODES + 127) // 128  # 79 source slabs
NPAD = NSLAB * 128  # 10112
AGRP = 16  # A tiles (slabs) per DMA
XGRP = 16  # x slabs per load chunk


def build_tables(src, dst, x, W, b):
    """Per-core packed adjacency + pre-slabbed x and other device inputs."""
    src = np.asarray(src).astype(np.int64)
    dst = np.asarray(dst).astype(np.int64)

    cnt = np.zeros((NPAD, NCORES * NPC), np.uint8)
    np.add.at(cnt, (src, dst), 1)

    xs_pad = np.zeros((NPAD, D), np.float32)
    xs_pad[:N_NODES] = np.asarray(x, np.float32)
    # xs[p, s*128 + f] = x[s*128 + p, f]
    xs = np.ascontiguousarray(
        xs_pad.reshape(NSLAB, 128, D).transpose(1, 0, 2).reshape(128, NSLAB * D)
    ).astype(ml_dtypes.bfloat16)

    assert cnt.max() <= 15, "edge multiplicity too large for fp8e4"
    adjs = []
    for c in range(NCORES):
        ac = cnt[:, c * NPC : (c + 1) * NPC].astype(ml_dtypes.float8_e4m3)
        ac = ac.reshape(NSLAB, 128, NPC)
        parts = []
        for ws, ww in WINDOWS:
            for s0 in range(0, NSLAB, AGRP):
                ng = min(AGRP, NSLAB - s0)
                # per-partition contiguous: [p][g][w]
                blk = ac[s0 : s0 + ng, :, ws : ws + ww].transpose(1, 0, 2)
                parts.append(np.ascontiguousarray(blk).reshape(-1))
        adjs.append(np.concatenate(parts))

    wt = np.asarray(W, np.float32).T.astype(ml_dtypes.bfloat16)  # [fi, fo]
    bb = np.asarray(b, np.float32).reshape(128, 1)
    return xs, adjs, wt, bb


def build_program():
    import concourse.mybir as mybir
    import concourse.tile as tile
    from concourse import bacc

    nc = bacc.Bacc("TRN2", target_bir_lowering=False, num_devices=NCORES)
    bf16 = mybir.dt.bfloat16
    f32 = mybir.dt.float32

    adj_len = sum(128 * ww for _, ww in WINDOWS) * NSLAB
    xs_d = nc.declare_dram_parameter("xs", [128, NSLAB * D], bf16, isOutput=False)
    adj_d = nc.declare_dram_parameter("adj", [adj_len], mybir.dt.float8e4, isOutput=False)
    wt_d = nc.declare_dram_parameter("wt", [128, 128], bf16, isOutput=False)
    b_d = nc.declare_dram_parameter("bias", [128, 1], f32, isOutput=False)
    out_d = nc.declare_dram_parameter("out", [128, NPC], f32, isOutput=True)

    with tile.TileContext(nc) as tc:
        with (
            tc.tile_pool(name="const", bufs=1) as const_pool,
            tc.tile_pool(name="adj", bufs=4) as adj_pool,
            tc.tile_pool(name="hbuf", bufs=2) as hbuf_pool,
            tc.tile_pool(name="obuf", bufs=2) as obuf_pool,
            tc.tile_pool(name="ph", bufs=2, space="PSUM") as ph_pool,
            tc.tile_pool(name="po", bufs=2, space="PSUM") as po_pool,
        ):
            xsc = []
            for k in range(0, NSLAB, XGRP):
                nk = min(XGRP, NSLAB - k)
                t = const_pool.tile([128, XGRP * D], bf16, tag=f"xs{k}")
                nc.scalar.dma_start(t[:, 0 : nk * D], xs_d[:, k * D : (k + nk) * D])
                xsc.append(t)
            wt_sb = const_pool.tile([128, 128], bf16)
            nc.sync.dma_start(wt_sb[:], wt_d[:])
            b_sb = const_pool.tile([128, 1], f32)
            nc.sync.dma_start(b_sb[:], b_d[:])

            off = 0
            for ws, ww in WINDOWS:
                ph = ph_pool.tile([128, ww], f32)
                at = None
                for s in range(NSLAB):
                    g = s % AGRP
                    if g == 0:
                        ng = min(AGRP, NSLAB - s)
                        at = adj_pool.tile([128, AGRP, ww], mybir.dt.float8e4)
                        nc.sync.dma_start(
                            at[:, 0:ng, :],
                            adj_d[off : off + ng * 128 * ww].rearrange(
                                "(p g w) -> p g w", g=ng, w=ww
                            ),
                        )
                        off += ng * 128 * ww
                    nc.tensor.matmul(
                        ph[:, :],
                        lhsT=xsc[s // XGRP][:, (s % XGRP) * D : (s % XGRP + 1) * D],
                        rhs=at[:, g, :],
                        start=(s == 0),
                        stop=(s == NSLAB - 1),
                    )
                hb = hbuf_pool.tile([128, ww], bf16)
                nc.vector.tensor_copy(hb[:], ph[:, :])
                po = po_pool.tile([128, ww], f32)
                nc.tensor.matmul(po[:, :], lhsT=wt_sb[:], rhs=hb[:], start=True, stop=True)
                ob = obuf_pool.tile([128, ww], f32)
                nc.vector.tensor_scalar_add(ob[:], po[:, :], b_sb[:, 0:1])
                nc.sync.dma_start(out_d[:, ws : ws + ww], ob[:])

    nc.compile()
    return nc


_CACHED_NC = None


def kernel(x, src, dst, W, b):
    global _CACHED_NC
    from concourse.bass_utils import run_bass_kernel_spmd

    xs, adjs, wt, bb = build_tables(src, dst, x, W, b)
    if _CACHED_NC is None:
        _CACHED_NC = build_program()
    nc = _CACHED_NC

    in_maps = [
        {"xs": xs, "adj": adjs[c], "wt": wt, "bias": bb} for c in range(NCORES)
    ]
    res = run_bass_kernel_spmd(nc, in_maps, list(range(NCORES)))
    out_t = np.concatenate([res.results[c]["out"] for c in range(NCORES)], axis=1)
    return np.ascontiguousarray(out_t.T[:N_NODES]).astype(np.float32)


# revision 15
# speedup vs baseline: 14.0918x; 1.0075x over previous
"""GCN layer (gather -> segment-sum -> linear) on 8 TRN2 NeuronCores.

out = segment_sum(x[src], dst, N) @ W.T + b

Strategy (dst-sharded SPMD, dense-adjacency matmul):
- The message-passing step is h = A @ x with A[d, s] = #edges s->d.
  Equivalently h^T = x^T @ A^T: contract over source nodes on TensorE.
- 10000 destinations split 1250 per core. Per core the host builds the
  dense fp8e4 adjacency block [10112 src, 1250 dst] (edge counts are small
  ints, exact in fp8e4) packed in the exact tile order the device streams:
  3 PSUM windows (512/512/226 dst) x 79 source slabs of 128, so every DMA
  is a contiguous line-rate read.
- Device: x lives in SBUF (pre-slabbed [128, 79*128] bf16, chunked loads
  interleaved with the adjacency stream on one HWDGE FIFO); per window, 79
  matmuls accumulate h^T[fi, dst] into a PSUM bank (lhsT = x slab
  [128 src, 128 fi] stationary bf16, rhs = A tile [128 src, ww] fp8);
  then h^T -> SBUF bf16, out^T = W @ h^T (bf16 matmul) + bias, DMA out.
- Host concatenates the per-core [128, 1250] outputs and transposes.
"""

import numpy as np
import ml_dtypes

N_NODES = 10000
D = 128
NCORES = 8
NPC = 1250  # dst nodes per core
WINDOWS = [(0, 512), (512, 512), (1024, 226)]  # (start, width) within a core
NSLAB = (N_NODES + 127) // 128  # 79 source slabs
NPAD = NSLAB * 128  # 10112
XGRP = 8  # x slabs per load chunk


def _groups(wi):
    """A-tile DMA group sizes (slabs per DMA) for window wi; ramped at the
    start of window 0 so the first matmul's data lands early."""
    ramp = [4, 8] if wi == 0 else []
    left = NSLAB - sum(ramp)
    out = list(ramp)
    while left > 0:
        g = min(16, left)
        out.append(g)
        left -= g
    return out


def build_tables(src, dst, x, W, b):
    """Per-core packed adjacency + pre-slabbed x and other device inputs."""
    src = np.asarray(src).astype(np.int64)
    dst = np.asarray(dst).astype(np.int64)

    cnt = np.zeros((NPAD, NCORES * NPC), np.uint8)
    np.add.at(cnt, (src, dst), 1)
    assert cnt.max() <= 15, "edge multiplicity too large for fp8e4"

    xs_pad = np.zeros((NPAD, D), np.float32)
    xs_pad[:N_NODES] = np.asarray(x, np.float32)
    # xs[p, s*128 + f] = x[s*128 + p, f]
    xs = np.ascontiguousarray(
        xs_pad.reshape(NSLAB, 128, D).transpose(1, 0, 2).reshape(128, NSLAB * D)
    ).astype(ml_dtypes.bfloat16)

    adjs = []
    for c in range(NCORES):
        ac = cnt[:, c * NPC : (c + 1) * NPC].astype(ml_dtypes.float8_e4m3)
        ac = ac.reshape(NSLAB, 128, NPC)
        parts = []
        for wi, (ws, ww) in enumerate(WINDOWS):
            s0 = 0
            for ng in _groups(wi):
                # per-partition contiguous: [p][g][w]
                blk = ac[s0 : s0 + ng, :, ws : ws + ww].transpose(1, 0, 2)
                parts.append(np.ascontiguousarray(blk).reshape(-1))
                s0 += ng
        adjs.append(np.concatenate(parts))

    wt = np.asarray(W, np.float32).T.astype(ml_dtypes.bfloat16)  # [fi, fo]
    bb = np.asarray(b, np.float32).reshape(128, 1)
    return xs, adjs, wt, bb


def build_program():
    import concourse.mybir as mybir
    import concourse.tile as tile
    from concourse import bacc

    nc = bacc.Bacc("TRN2", target_bir_lowering=False, num_devices=NCORES)
    bf16 = mybir.dt.bfloat16
    f32 = mybir.dt.float32
    fp8 = mybir.dt.float8e4

    adj_len = sum(128 * ww for _, ww in WINDOWS) * NSLAB
    nchunk = (NSLAB + XGRP - 1) // XGRP
    xs_d = nc.declare_dram_parameter("xs", [128, NSLAB * D], bf16, isOutput=False)
    adj_d = nc.declare_dram_parameter("adj", [adj_len], fp8, isOutput=False)
    wt_d = nc.declare_dram_parameter("wt", [128, 128], bf16, isOutput=False)
    b_d = nc.declare_dram_parameter("bias", [128, 1], f32, isOutput=False)
    out_d = nc.declare_dram_parameter("out", [128, NPC], f32, isOutput=True)

    with tile.TileContext(nc) as tc:
        with (
            tc.tile_pool(name="const", bufs=1) as const_pool,
            tc.tile_pool(name="adj", bufs=4) as adj_pool,
            tc.tile_pool(name="hbuf", bufs=2) as hbuf_pool,
            tc.tile_pool(name="obuf", bufs=2) as obuf_pool,
            tc.tile_pool(name="ph", bufs=2, space="PSUM") as ph_pool,
            tc.tile_pool(name="po", bufs=2, space="PSUM") as po_pool,
        ):
            # x chunks: interleaved with the A stream on the same HWDGE FIFO
            # so the first matmul's inputs land first and later chunks pace in.
            xsc = [
                const_pool.tile([128, XGRP * D], bf16, tag=f"xs{k}", name=f"xs{k}")
                for k in range(nchunk)
            ]

            def load_chunk(k):
                lo, hi = k * XGRP, min((k + 1) * XGRP, NSLAB)
                nc.sync.dma_start(
                    xsc[k][:, 0 : (hi - lo) * D], xs_d[:, lo * D : hi * D]
                )

            wt_sb = const_pool.tile([128, 128], bf16)
            nc.scalar.dma_start(wt_sb[:], wt_d[:])
            b_sb = const_pool.tile([128, 1], f32)
            nc.scalar.dma_start(b_sb[:], b_d[:])

            load_chunk(0)
            next_chunk = 1
            off = 0
            for wi, (ws, ww) in enumerate(WINDOWS):
                ph = ph_pool.tile([128, ww], f32)
                s = 0
                for ng in _groups(wi):
                    at = adj_pool.tile([128, 16, ww], fp8, tag="adj")
                    nc.sync.dma_start(
                        at[:, 0:ng, :],
                        adj_d[off : off + ng * 128 * ww].rearrange(
                            "(p g w) -> p g w", g=ng, w=ww
                        ),
                    )
                    off += ng * 128 * ww
                    # everything the upcoming matmuls read must be issued first
                    while next_chunk < nchunk and next_chunk * XGRP < s + ng:
                        load_chunk(next_chunk)
                        next_chunk += 1
                    for g in range(ng):
                        nc.tensor.matmul(
                            ph[:, :],
                            lhsT=xsc[(s + g) // XGRP][
                                :, ((s + g) % XGRP) * D : ((s + g) % XGRP + 1) * D
                            ],
                            rhs=at[:, g, :],
                            start=(s + g == 0),
                            stop=(s + g == NSLAB - 1),
                        )
                    s += ng
                hb = hbuf_pool.tile([128, ww], bf16)
                nc.vector.tensor_copy(hb[:], ph[:, :])
                po = po_pool.tile([128, ww], f32)
                nc.tensor.matmul(po[:, :], lhsT=wt_sb[:], rhs=hb[:], start=True, stop=True)
                ob = obuf_pool.tile([128, ww], f32)
                nc.vector.tensor_scalar_add(ob[:], po[:, :], b_sb[:, 0:1])
                nc.scalar.dma_start(out_d[:, ws : ws + ww], ob[:])

    nc.compile()
    return nc


_CACHED_NC = None


def kernel(x, src, dst, W, b):
    global _CACHED_NC
    from concourse.bass_utils import run_bass_kernel_spmd

    xs, adjs, wt, bb = build_tables(src, dst, x, W, b)
    if _CACHED_NC is None:
        _CACHED_NC = build_program()
    nc = _CACHED_NC

    in_maps = [
        {"xs": xs, "adj": adjs[c], "wt": wt, "bias": bb} for c in range(NCORES)
    ]
    res = run_bass_kernel_spmd(nc, in_maps, list(range(NCORES)))
    out_t = np.concatenate([res.results[c]["out"] for c in range(NCORES)], axis=1)
    return np.ascontiguousarray(out_t.T[:N_NODES]).astype(np.float32)


# revision 16
# speedup vs baseline: 14.4924x; 1.0284x over previous
"""GCN layer (gather -> segment-sum -> linear) on 8 TRN2 NeuronCores.

out = segment_sum(x[src], dst, N) @ W.T + b

Strategy (dst-sharded SPMD, dense-adjacency matmul):
- The message-passing step is h = A @ x with A[d, s] = #edges s->d.
  Equivalently h^T = x^T @ A^T: contract over source nodes on TensorE.
- 10000 destinations split 1250 per core. Per core the host builds the
  dense fp8e4 adjacency block [10112 src, 1250 dst] (edge counts are small
  ints, exact in fp8e4) packed in the exact tile order the device streams:
  3 PSUM windows (512/512/226 dst) x 79 source slabs of 128, so every DMA
  is a contiguous line-rate read.
- Device: x lives in SBUF (pre-slabbed [128, 79*128] bf16, chunked loads
  interleaved with the adjacency stream on one HWDGE FIFO); per window, 79
  matmuls accumulate h^T[fi, dst] into a PSUM bank (lhsT = x slab
  [128 src, 128 fi] stationary bf16, rhs = A tile [128 src, ww] fp8);
  then h^T -> SBUF bf16, out^T = W @ h^T (bf16 matmul) + bias, DMA out.
- Host concatenates the per-core [128, 1250] outputs and transposes.
"""

import numpy as np
import ml_dtypes

N_NODES = 10000
D = 128
NCORES = 8
NPC = 1250  # dst nodes per core
WINDOWS = [(0, 512), (512, 512), (1024, 226)]  # (start, width) within a core
NSLAB = (N_NODES + 127) // 128  # 79 source slabs
NPAD = NSLAB * 128  # 10112
XGRP = 8  # x slabs per load chunk


def _groups(wi):
    """A-tile DMA group sizes (slabs per DMA) for window wi; ramped at the
    start of window 0 so the first matmul's data lands early."""
    ramp = [2, 4, 8] if wi == 0 else []
    left = NSLAB - sum(ramp)
    out = list(ramp)
    while left > 0:
        g = min(16, left)
        out.append(g)
        left -= g
    return out


def build_tables(src, dst, x, W, b):
    """Per-core packed adjacency + pre-slabbed x and other device inputs."""
    src = np.asarray(src).astype(np.int64)
    dst = np.asarray(dst).astype(np.int64)

    cnt = np.zeros((NPAD, NCORES * NPC), np.uint8)
    np.add.at(cnt, (src, dst), 1)
    assert cnt.max() <= 15, "edge multiplicity too large for fp8e4"

    xs_pad = np.zeros((NPAD, D), np.float32)
    xs_pad[:N_NODES] = np.asarray(x, np.float32)
    # xs[p, s*128 + f] = x[s*128 + p, f]
    xs = np.ascontiguousarray(
        xs_pad.reshape(NSLAB, 128, D).transpose(1, 0, 2).reshape(128, NSLAB * D)
    ).astype(ml_dtypes.bfloat16)

    adjs = []
    for c in range(NCORES):
        ac = cnt[:, c * NPC : (c + 1) * NPC].astype(ml_dtypes.float8_e4m3)
        ac = ac.reshape(NSLAB, 128, NPC)
        parts = []
        for wi, (ws, ww) in enumerate(WINDOWS):
            s0 = 0
            for ng in _groups(wi):
                # per-partition contiguous: [p][g][w]
                blk = ac[s0 : s0 + ng, :, ws : ws + ww].transpose(1, 0, 2)
                parts.append(np.ascontiguousarray(blk).reshape(-1))
                s0 += ng
        adjs.append(np.concatenate(parts))

    wt = np.asarray(W, np.float32).T.astype(ml_dtypes.bfloat16)  # [fi, fo]
    bb = np.asarray(b, np.float32).reshape(128, 1)
    return xs, adjs, wt, bb


def build_program():
    import concourse.mybir as mybir
    import concourse.tile as tile
    from concourse import bacc

    nc = bacc.Bacc("TRN2", target_bir_lowering=False, num_devices=NCORES)
    bf16 = mybir.dt.bfloat16
    f32 = mybir.dt.float32
    fp8 = mybir.dt.float8e4

    adj_len = sum(128 * ww for _, ww in WINDOWS) * NSLAB
    nchunk = (NSLAB + XGRP - 1) // XGRP
    xs_d = nc.declare_dram_parameter("xs", [128, NSLAB * D], bf16, isOutput=False)
    adj_d = nc.declare_dram_parameter("adj", [adj_len], fp8, isOutput=False)
    wt_d = nc.declare_dram_parameter("wt", [128, 128], bf16, isOutput=False)
    b_d = nc.declare_dram_parameter("bias", [128, 1], f32, isOutput=False)
    out_d = nc.declare_dram_parameter("out", [128, NPC], f32, isOutput=True)

    with tile.TileContext(nc) as tc:
        with (
            tc.tile_pool(name="const", bufs=1) as const_pool,
            tc.tile_pool(name="adj", bufs=6) as adj_pool,
            tc.tile_pool(name="hbuf", bufs=2) as hbuf_pool,
            tc.tile_pool(name="obuf", bufs=2) as obuf_pool,
            tc.tile_pool(name="ph", bufs=2, space="PSUM") as ph_pool,
            tc.tile_pool(name="po", bufs=2, space="PSUM") as po_pool,
        ):
            # x chunks: interleaved with the A stream on the same HWDGE FIFO
            # so the first matmul's inputs land first and later chunks pace in.
            xsc = [
                const_pool.tile([128, XGRP * D], bf16, tag=f"xs{k}", name=f"xs{k}")
                for k in range(nchunk)
            ]

            def load_chunk(k):
                lo, hi = k * XGRP, min((k + 1) * XGRP, NSLAB)
                nc.sync.dma_start(
                    xsc[k][:, 0 : (hi - lo) * D], xs_d[:, lo * D : hi * D]
                )

            wt_sb = const_pool.tile([128, 128], bf16)
            nc.scalar.dma_start(wt_sb[:], wt_d[:])
            b_sb = const_pool.tile([128, 1], f32)
            nc.scalar.dma_start(b_sb[:], b_d[:])

            load_chunk(0)
            next_chunk = 1
            off = 0
            for wi, (ws, ww) in enumerate(WINDOWS):
                ph = ph_pool.tile([128, ww], f32)
                s = 0
                for ng in _groups(wi):
                    at = adj_pool.tile([128, 16, ww], fp8, tag="adj")
                    nc.sync.dma_start(
                        at[:, 0:ng, :],
                        adj_d[off : off + ng * 128 * ww].rearrange(
                            "(p g w) -> p g w", g=ng, w=ww
                        ),
                    )
                    off += ng * 128 * ww
                    # everything the upcoming matmuls read must be issued first
                    while next_chunk < nchunk and next_chunk * XGRP < s + ng:
                        load_chunk(next_chunk)
                        next_chunk += 1
                    for g in range(ng):
                        nc.tensor.matmul(
                            ph[:, :],
                            lhsT=xsc[(s + g) // XGRP][
                                :, ((s + g) % XGRP) * D : ((s + g) % XGRP + 1) * D
                            ],
                            rhs=at[:, g, :],
                            start=(s + g == 0),
                            stop=(s + g == NSLAB - 1),
                        )
                    s += ng
                hb = hbuf_pool.tile([128, ww], bf16)
                nc.vector.tensor_copy(hb[:], ph[:, :])
                po = po_pool.tile([128, ww], f32)
                nc.tensor.matmul(po[:, :], lhsT=wt_sb[:], rhs=hb[:], start=True, stop=True)
                ob = obuf_pool.tile([128, ww], f32)
                nc.vector.tensor_scalar_add(ob[:], po[:, :], b_sb[:, 0:1])
                nc.scalar.dma_start(out_d[:, ws : ws + ww], ob[:])

    nc.compile()
    return nc


_CACHED_NC = None


def kernel(x, src, dst, W, b):
    global _CACHED_NC
    from concourse.bass_utils import run_bass_kernel_spmd

    xs, adjs, wt, bb = build_tables(src, dst, x, W, b)
    if _CACHED_NC is None:
        _CACHED_NC = build_program()
    nc = _CACHED_NC

    in_maps = [
        {"xs": xs, "adj": adjs[c], "wt": wt, "bias": bb} for c in range(NCORES)
    ]
    res = run_bass_kernel_spmd(nc, in_maps, list(range(NCORES)))
    out_t = np.concatenate([res.results[c]["out"] for c in range(NCORES)], axis=1)
    return np.ascontiguousarray(out_t.T[:N_NODES]).astype(np.float32)
